# revision 28
# baseline (speedup 1.0000x reference)
"""Trainium2 Bass kernel for nn_AblationRouter (moe_routing).

Computation (per batch row):
  h = EMA(x) with per-channel decay beta (constant 0.9 here)
  hid = relu([x, h] @ W1^T + b1);  route = hid @ W2^T + b2
  gates = softmax(route @ Wr^T + br)

Strategy: data-parallel over B=8 batch rows, one per NeuronCore.
W2 and Wr are both linear with no nonlinearity between them, so they are
folded on the host into Wc = Wr @ W2 [E, H] (and bc = br + Wr @ b2),
which removes the route matmul entirely: logits = hid @ Wc^T + bc.

The EMA recurrence is computed as matmuls against a precomputed decay
matrix over 512-token blocks with a 128-token lookback (beta^128 ~ 1e-6,
negligible vs matmul rounding -> no serial carry chain); the decay
matrix is banded so each 128-row s-chunk only streams the token range it
can reach.  All activations are channel-major ([channel, token]) so the
matmuls chain on the TensorEngine without transposes.

M1 (the dominant matmul, [x,h][2048] -> hid[4096]) optionally runs the
first NF8 of 8 x-channel tiles in fp8 e4m3 DoubleRow mode (2x PE
throughput); the rest stays fp16.  All M1 operands carry a uniform
2^10 product scale (x*16 folded into the host transpose, h*16 folded
into the decay matrix, W1*64 folded on host) so fp8 values sit in
e4m3's normal range; the descale 2^-10 rides the relu activation.
"""

import sys

if "/opt/trn_rl_repo" not in sys.path:
    sys.path.insert(0, "/opt/trn_rl_repo")

import numpy as np
import ml_dtypes

# Problem shapes (hardcoded per harness contract)
B, T, D, E, H = 8, 2048, 1024, 64, 4096
TP = T  # padded token count (2047 real + 1 pad)
BLK = 512  # token block (psum free-dim)
LB = 128  # lookback tokens
SC = (LB + BLK) // 128  # 5 s-chunks per block
NBLK = TP // BLK  # 4 blocks
NDT = D // 128  # 8 d-tiles
NHT = H // 128  # 32 hid-tiles
NKT = (2 * D) // 128  # 16 k-tiles for M1
BPW = 2  # 512-blocks per weight-streaming window
NW = NBLK // BPW
WTOK = BPW * BLK  # tokens per window

NF8 = 8  # x-channel d-tiles (of 8) quantized to fp8 e4m3 (must be even)

XS = 16.0  # activation scale 2^4 (both x and h)
WS = 64.0  # W1 scale 2^6
DESCALE = 1.0 / (XS * WS)


def _build_program(nf8=NF8):
    import concourse.bacc as bacc
    import concourse.mybir as mybir
    import concourse.tile as tile
    from concourse._compat import axon_active

    f32 = mybir.dt.float32
    f16 = mybir.dt.float16
    f8 = mybir.dt.float8e4
    AF = mybir.ActivationFunctionType
    AX = mybir.AxisListType
    DR = mybir.MatmulPerfMode.DoubleRow

    nc = bacc.Bacc("TRN2", target_bir_lowering=False, debug=not axon_active())

    # --- DRAM I/O ---
    x_pad = nc.dram_tensor("x_pad", [LB + TP, D], f16, kind="ExternalInput")
    mdec = nc.dram_tensor("mdec", [SC * 128, BLK], f16, kind="ExternalInput")
    if nf8:
        xt8 = nc.dram_tensor("xt8", [nf8 * 128, TP], f8, kind="ExternalInput")
        w1t8 = nc.dram_tensor("w1t8", [NHT, 128, nf8 * 128], f8, kind="ExternalInput")
    if nf8 < NDT:
        xt16 = nc.dram_tensor(
            "xt16", [(NDT - nf8) * 128, TP], f16, kind="ExternalInput"
        )
    w1t16 = nc.dram_tensor(
        "w1t16", [NHT, 128, (NKT - nf8) * 128], f16, kind="ExternalInput"
    )
    b1t = nc.dram_tensor("b1t", [128, NHT], f32, kind="ExternalInput")
    wct = nc.dram_tensor("wct", [128, NHT * E], f16, kind="ExternalInput")
    bcb = nc.dram_tensor("bcb", [128, (BLK // 128) * E], f32, kind="ExternalInput")
    out = nc.dram_tensor("out", [TP, E], f32, kind="ExternalOutput")

    with tile.TileContext(nc) as tc:
        with (
            tc.tile_pool(name="const", bufs=1) as cpool,
            tc.tile_pool(name="xin", bufs=1) as xpool,
            tc.tile_pool(name="xtin", bufs=1) as xtpool,
            tc.tile_pool(name="acts", bufs=1) as apool,
            tc.tile_pool(name="w1", bufs=4) as w1pool,
            tc.tile_pool(name="sm", bufs=2) as smpool,
            tc.tile_pool(name="ema_ps", bufs=2, space="PSUM") as ema_ps,
            tc.tile_pool(name="m1_ps", bufs=3, space="PSUM") as m1_ps,
            tc.tile_pool(name="m3_ps", bufs=2, space="PSUM") as m3_ps,
        ):
            # Constants: decay matrix first (first EMA chain blocks on it);
            # b1/wc/bc are only read much later, issue them after the first
            # window's input stream below.
            m_sb = cpool.tile([128, SC * BLK], f16, tag="mdec")
            for sc in range(SC):
                nc.sync.dma_start(
                    m_sb[:, sc * BLK : (sc + 1) * BLK],
                    mdec[sc * 128 : (sc + 1) * 128, :],
                )
            b1_sb = cpool.tile([128, NHT], f32, tag="b1")
            wc_sb = cpool.tile([128, NHT * E], f16, tag="wc")
            bc_sb = cpool.tile([128, (BLK // 128) * E], f32, tag="bc")

            NSR = LB // 128 + BPW * (BLK // 128)  # x rows (128-chunks) per window
            for w in range(NW):
                t0 = w * WTOK
                # --- stream inputs for this window ---
                x_sb = xpool.tile([128, NSR * D], f16, tag="x")
                for sc in range(NSR):
                    nc.sync.dma_start(
                        x_sb[:, sc * D : (sc + 1) * D],
                        x_pad[t0 + sc * 128 : t0 + (sc + 1) * 128, :],
                    )
                if nf8:
                    xt8_sb = xtpool.tile([128, nf8 * WTOK], f8, tag="xt8")
                    for dt in range(nf8):
                        nc.sync.dma_start(
                            xt8_sb[:, dt * WTOK : (dt + 1) * WTOK],
                            xt8[dt * 128 : (dt + 1) * 128, t0 : t0 + WTOK],
                        )
                if nf8 < NDT:
                    xt16_sb = xtpool.tile([128, (NDT - nf8) * WTOK], f16, tag="xt16")
                    for dt in range(NDT - nf8):
                        nc.sync.dma_start(
                            xt16_sb[:, dt * WTOK : (dt + 1) * WTOK],
                            xt16[dt * 128 : (dt + 1) * 128, t0 : t0 + WTOK],
                        )
                if w == 0:
                    nc.sync.dma_start(b1_sb[:], b1t[:])
                    nc.sync.dma_start(wc_sb[:], wct[:])
                    nc.sync.dma_start(bc_sb[:], bcb[:])

                # --- EMA -> hT (channel-major, = 16*h in fp16).  beta^96~4e-5
                # so each s-chunk only streams the ~96-token range it can
                # reach; chunk 1 goes first at full width to zero-init the
                # whole PSUM tile (its out-of-band matrix entries are 0). ---
                EMA_PLAN = [
                    (1, 0, BLK, True),
                    (0, 0, 96, False),
                    (2, 128, 352, False),
                    (3, 256, 480, False),
                    (4, 384, BLK, False),
                ]
                ht_sb = apool.tile([128, NDT * WTOK], f16, tag="ht")
                for blk in range(BPW):
                    for dt in range(NDT):
                        ps = ema_ps.tile([128, BLK], f32, tag="ema")
                        for i, (sc, lo, hi, st) in enumerate(EMA_PLAN):
                            off = (blk * (BLK // 128) + sc) * D
                            nc.tensor.matmul(
                                ps[:, lo:hi],
                                x_sb[:, off + dt * 128 : off + (dt + 1) * 128],
                                m_sb[:, sc * BLK + lo : sc * BLK + hi],
                                start=st,
                                stop=(i == len(EMA_PLAN) - 1),
                            )
                        nc.vector.tensor_copy(
                            ht_sb[:, dt * WTOK + blk * BLK : dt * WTOK + (blk + 1) * BLK],
                            ps[:],
                        )

                # --- M1 (blk-major so each block's M2'+softmax overlaps the
                # next block's M1) + M2' + softmax ---
                hid_sb = apool.tile([128, NHT * WTOK], f16, tag="hid")
                for blk in range(BPW):
                    for ht in range(NHT):
                        if nf8:
                            w18_sb = w1pool.tile([128, nf8 * 128], f8, tag="w18")
                            nc.sync.dma_start(w18_sb[:], w1t8[ht, :, :])
                        w116_sb = w1pool.tile(
                            [128, (NKT - nf8) * 128], f16, tag="w116"
                        )
                        half = (NKT - nf8) * 64
                        nc.sync.dma_start(w116_sb[:, :half], w1t16[ht, :, :half])
                        nc.sync.dma_start(w116_sb[:, half:], w1t16[ht, :, half:])
                        ps1 = m1_ps.tile([128, BLK], f32, tag="m1")
                        nmm = nf8 // 2 + (NKT - nf8)
                        mi = 0
                        for j in range(nf8 // 2):
                            nc.tensor.matmul(
                                ps1[:],
                                w18_sb[:, j * 256 : (j + 1) * 256].rearrange(
                                    "p (two m) -> p two m", m=128
                                ),
                                xt8_sb[:, 2 * j * WTOK : (2 * j + 2) * WTOK]
                                .rearrange("p (dt w) -> p dt w", w=WTOK)[
                                    :, :, blk * BLK : (blk + 1) * BLK
                                ],
                                start=(mi == 0),
                                stop=(mi == nmm - 1),
                                perf_mode=DR,
                            )
                            mi += 1
                        for kt in range(nf8, NKT):
                            src = xt16_sb if kt < NDT else ht_sb
                            doff = ((kt - nf8) if kt < NDT else (kt - NDT)) * WTOK
                            nc.tensor.matmul(
                                ps1[:],
                                w116_sb[:, (kt - nf8) * 128 : (kt - nf8 + 1) * 128],
                                src[:, doff + blk * BLK : doff + (blk + 1) * BLK],
                                start=(mi == 0),
                                stop=(mi == nmm - 1),
                            )
                            mi += 1
                        nc.scalar.activation(
                            hid_sb[
                                :, ht * WTOK + blk * BLK : ht * WTOK + (blk + 1) * BLK
                            ],
                            ps1[:],
                            AF.Relu,
                            bias=b1_sb[:, ht : ht + 1],
                            scale=DESCALE,
                        )

                    # --- M2': logits = hid @ Wc^T + bc, tokens on partitions,
                    # then softmax.  Logits are O(1) here so exp runs without
                    # the max-subtraction; sum/recip on Vector, exp + the
                    # final normalize (Copy with scale=rcp) on Scalar. ---
                    bt0 = t0 + blk * BLK
                    ot = smpool.tile([128, (BLK // 128) * E], f32, tag="ot")
                    for tt in range(BLK // 128):
                        ps3 = m3_ps.tile([128, E], f32, tag="m3")
                        for ht in range(NHT):
                            hoff = ht * WTOK + blk * BLK + tt * 128
                            nc.tensor.matmul(
                                ps3[:],
                                hid_sb[:, hoff : hoff + 128],
                                wc_sb[:, ht * E : (ht + 1) * E],
                                start=(ht == 0),
                                stop=(ht == NHT - 1),
                            )
                        lg = smpool.tile([128, E], f32, tag="lg")
                        nc.vector.tensor_add(lg[:], ps3[:], bc_sb[:, :E])
                        ex = smpool.tile([128, E], f32, tag="ex")
                        nc.scalar.activation(ex[:], lg[:], AF.Exp)
                        ssum = smpool.tile([128, 1], f32, tag="ssum")
                        nc.vector.reduce_sum(ssum[:], ex[:], axis=AX.X)
                        rcp = smpool.tile([128, 1], f32, tag="rcp")
                        nc.vector.reciprocal(rcp[:], ssum[:])
                        nc.scalar.activation(
                            ot[:, tt * E : (tt + 1) * E], ex[:], AF.Copy, scale=rcp[:]
                        )
                    # single DMA per block: [4 tok-tiles, 128, E]
                    nc.sync.dma_start(
                        out[bt0 : bt0 + BLK, :].rearrange("(tt p) e -> p tt e", p=128),
                        ot[:].rearrange("p (tt e) -> p tt e", e=E),
                    )

    nc.compile()
    return nc


_prepared = {}


def _prepare_host_inputs(seq, beta_raw, W1, b1, W2, b2, Wr, br, nf8=NF8):
    f8np = ml_dtypes.float8_e4m3
    seq = np.asarray(seq, np.float32)
    beta = 1.0 / (1.0 + np.exp(-np.asarray(beta_raw, np.float64)))
    assert beta.max() - beta.min() < 1e-6, "kernel assumes channel-constant beta"
    b = float(beta[0])
    assert b ** LB < 1e-4, "lookback too short for this beta"

    x = seq[:, : T - 1, :]  # [B, 2047, D]

    # decay matrix: mdec[s, t] = b^((t+LB)-s) for (t+LB)>=s else 0; carries the
    # 2^4 h-scale so the EMA output lands pre-scaled for M1
    s_idx = np.arange(LB + BLK)[:, None]
    t_idx = np.arange(BLK)[None, :]
    expo = (t_idx + LB) - s_idx
    mdec = (XS * np.where(expo >= 0, b ** np.maximum(expo, 0), 0.0)).astype(np.float16)

    W1s = np.asarray(W1, np.float32) * WS
    W2 = np.asarray(W2, np.float32)
    Wr = np.asarray(Wr, np.float32)
    # fold router into predictor layer 2: logits = hid @ (Wr@W2)^T + (br + Wr@b2)
    Wc = (Wr @ W2).astype(np.float32)  # [E, H]
    bc_eff = np.asarray(br, np.float32) + Wr @ np.asarray(b2, np.float32)

    # w1t8[ht, k, j*256 + i*128 + m] = W1s[ht*128+m, (2j+i)*128+k] (DoubleRow pairs)
    if nf8:
        w1x = W1s[:, : nf8 * 128].reshape(NHT, 128, nf8 // 2, 2, 128)
        w1t8 = np.ascontiguousarray(
            w1x.transpose(0, 4, 2, 3, 1).reshape(NHT, 128, nf8 * 128)
        ).astype(f8np)
    # w1t16[ht, k, c*128+m] = W1s[ht*128+m, (nf8+c)*128+k]
    w1r = W1s.reshape(NHT, 128, NKT, 128)[:, :, nf8:, :]
    w1t16 = np.ascontiguousarray(
        w1r.transpose(0, 3, 2, 1).reshape(NHT, 128, (NKT - nf8) * 128)
    ).astype(np.float16)
    b1t = np.ascontiguousarray(np.asarray(b1, np.float32).reshape(NHT, 128).T)
    # wct[p, ht*E+e] = Wc[e, ht*128+p]
    wct = np.ascontiguousarray(
        Wc.T.reshape(NHT, 128, E).transpose(1, 0, 2).reshape(128, NHT * E)
    ).astype(np.float16)
    bcb = np.ascontiguousarray(np.tile(bc_eff[None, :], (128, BLK // 128)))

    shared = dict(mdec=mdec, w1t16=w1t16, b1t=b1t, wct=wct, bcb=bcb)
    if nf8:
        shared["w1t8"] = w1t8
    in_maps = []
    for bi in range(B):
        x_pad = np.zeros((LB + TP, D), np.float16)
        x_pad[LB : LB + T - 1] = x[bi]
        xTs = np.zeros((D, TP), np.float32)
        xTs[:, : T - 1] = x[bi].T * XS
        m = dict(shared)
        m["x_pad"] = x_pad
        if nf8:
            m["xt8"] = np.ascontiguousarray(xTs[: nf8 * 128]).astype(f8np)
        if nf8 < NDT:
            m["xt16"] = np.ascontiguousarray(xTs[nf8 * 128 :]).astype(np.float16)
        in_maps.append(m)
    return in_maps


def kernel(**inputs):
    from concourse import bass_utils

    if NF8 not in _prepared:
        _prepared[NF8] = _build_program()
    nc = _prepared[NF8]
    in_maps = _prepare_host_inputs(**inputs)
    res = bass_utils.run_bass_kernel_spmd(nc, in_maps, core_ids=list(range(B)))
    outs = np.stack([r["out"] for r in res.results], axis=0)  # [B, TP, E]
    return outs[:, : T - 1, :].astype(np.float32)


# revision 29
# speedup vs baseline: 1.0105x; 1.0105x over previous
"""Trainium2 Bass kernel for nn_AblationRouter (moe_routing).

Computation (per batch row):
  h = EMA(x) with per-channel decay beta (constant 0.9 here)
  hid = relu([x, h] @ W1^T + b1);  route = hid @ W2^T + b2
  gates = softmax(route @ Wr^T + br)

Strategy: data-parallel over B=8 batch rows, one per NeuronCore.
W2 and Wr are both linear with no nonlinearity between them, so they are
folded on the host into Wc = Wr @ W2 [E, H] (and bc = br + Wr @ b2),
which removes the route matmul entirely: logits = hid @ Wc^T + bc.

The EMA recurrence is computed as matmuls against a precomputed decay
matrix over 512-token blocks with a 128-token lookback (beta^128 ~ 1e-6,
negligible vs matmul rounding -> no serial carry chain); the decay
matrix is banded so each 128-row s-chunk only streams the token range it
can reach.  All activations are channel-major ([channel, token]) so the
matmuls chain on the TensorEngine without transposes.

M1 (the dominant matmul, [x,h][2048] -> hid[4096]) optionally runs the
first NF8 of 8 x-channel tiles in fp8 e4m3 DoubleRow mode (2x PE
throughput); the rest stays fp16.  All M1 operands carry a uniform
2^10 product scale (x*16 folded into the host transpose, h*16 folded
into the decay matrix, W1*64 folded on host) so fp8 values sit in
e4m3's normal range; the descale 2^-10 rides the relu activation.
"""

import sys

if "/opt/trn_rl_repo" not in sys.path:
    sys.path.insert(0, "/opt/trn_rl_repo")

import numpy as np
import ml_dtypes

# Problem shapes (hardcoded per harness contract)
B, T, D, E, H = 8, 2048, 1024, 64, 4096
TP = T  # padded token count (2047 real + 1 pad)
BLK = 512  # token block (psum free-dim)
LB = 128  # lookback tokens
SC = (LB + BLK) // 128  # 5 s-chunks per block
NBLK = TP // BLK  # 4 blocks
NDT = D // 128  # 8 d-tiles
NHT = H // 128  # 32 hid-tiles
NKT = (2 * D) // 128  # 16 k-tiles for M1
BPW = 2  # 512-blocks per weight-streaming window
NW = NBLK // BPW
WTOK = BPW * BLK  # tokens per window

NF8 = 8  # x-channel d-tiles (of 8) quantized to fp8 e4m3 (must be even)

XS = 16.0  # activation scale 2^4 (both x and h)
WS = 64.0  # W1 scale 2^6
DESCALE = 1.0 / (XS * WS)


def _build_program(nf8=NF8):
    import concourse.bacc as bacc
    import concourse.mybir as mybir
    import concourse.tile as tile
    from concourse._compat import axon_active

    f32 = mybir.dt.float32
    f16 = mybir.dt.float16
    f8 = mybir.dt.float8e4
    AF = mybir.ActivationFunctionType
    AX = mybir.AxisListType
    DR = mybir.MatmulPerfMode.DoubleRow

    nc = bacc.Bacc("TRN2", target_bir_lowering=False, debug=not axon_active())

    # --- DRAM I/O ---
    x_pad = nc.dram_tensor("x_pad", [LB + TP, D], f16, kind="ExternalInput")
    mdec = nc.dram_tensor("mdec", [SC * 128, BLK], f16, kind="ExternalInput")
    if nf8:
        xt8 = nc.dram_tensor("xt8", [nf8 * 128, TP], f8, kind="ExternalInput")
        w1t8 = nc.dram_tensor("w1t8", [NHT, 128, nf8 * 128], f8, kind="ExternalInput")
    if nf8 < NDT:
        xt16 = nc.dram_tensor(
            "xt16", [(NDT - nf8) * 128, TP], f16, kind="ExternalInput"
        )
    w1t16 = nc.dram_tensor(
        "w1t16", [NHT, 128, (NKT - nf8) * 128], f16, kind="ExternalInput"
    )
    b1t = nc.dram_tensor("b1t", [128, NHT], f32, kind="ExternalInput")
    wct = nc.dram_tensor("wct", [128, NHT * E], f16, kind="ExternalInput")
    bcb = nc.dram_tensor("bcb", [128, (BLK // 128) * E], f32, kind="ExternalInput")
    out = nc.dram_tensor("out", [TP, E], f32, kind="ExternalOutput")

    with tile.TileContext(nc) as tc:
        with (
            tc.tile_pool(name="const", bufs=1) as cpool,
            tc.tile_pool(name="xin", bufs=1) as xpool,
            tc.tile_pool(name="xtin", bufs=1) as xtpool,
            tc.tile_pool(name="acts", bufs=1) as apool,
            tc.tile_pool(name="w1", bufs=4) as w1pool,
            tc.tile_pool(name="sm", bufs=2) as smpool,
            tc.tile_pool(name="ema_ps", bufs=2, space="PSUM") as ema_ps,
            tc.tile_pool(name="m1_ps", bufs=3, space="PSUM") as m1_ps,
            tc.tile_pool(name="m3_ps", bufs=2, space="PSUM") as m3_ps,
        ):
            # Constants: decay matrix first (first EMA chain blocks on it);
            # b1/wc/bc are only read much later, issue them after the first
            # window's input stream below.
            m_sb = cpool.tile([128, SC * BLK], f16, tag="mdec")
            for sc in range(SC):
                nc.sync.dma_start(
                    m_sb[:, sc * BLK : (sc + 1) * BLK],
                    mdec[sc * 128 : (sc + 1) * 128, :],
                )
            b1_sb = cpool.tile([128, NHT], f32, tag="b1")
            wc_sb = cpool.tile([128, NHT * E], f16, tag="wc")
            bc_sb = cpool.tile([128, (BLK // 128) * E], f32, tag="bc")

            NSR = LB // 128 + BPW * (BLK // 128)  # x rows (128-chunks) per window
            for w in range(NW):
                t0 = w * WTOK
                # --- stream inputs for this window ---
                x_sb = xpool.tile([128, NSR * D], f16, tag="x")
                for sc in range(NSR):
                    nc.sync.dma_start(
                        x_sb[:, sc * D : (sc + 1) * D],
                        x_pad[t0 + sc * 128 : t0 + (sc + 1) * 128, :],
                    )
                if nf8:
                    xt8_sb = xtpool.tile([128, nf8 * WTOK], f8, tag="xt8")
                    for dt in range(nf8):
                        nc.sync.dma_start(
                            xt8_sb[:, dt * WTOK : (dt + 1) * WTOK],
                            xt8[dt * 128 : (dt + 1) * 128, t0 : t0 + WTOK],
                        )
                if nf8 < NDT:
                    xt16_sb = xtpool.tile([128, (NDT - nf8) * WTOK], f16, tag="xt16")
                    for dt in range(NDT - nf8):
                        nc.sync.dma_start(
                            xt16_sb[:, dt * WTOK : (dt + 1) * WTOK],
                            xt16[dt * 128 : (dt + 1) * 128, t0 : t0 + WTOK],
                        )
                if w == 0:
                    nc.sync.dma_start(b1_sb[:], b1t[:])
                    nc.sync.dma_start(wc_sb[:], wct[:])
                    nc.sync.dma_start(bc_sb[:], bcb[:])

                # --- EMA -> hT (channel-major, = 16*h in fp16).  beta^96~4e-5
                # so each s-chunk only streams the ~96-token range it can
                # reach; chunk 1 goes first at full width to zero-init the
                # whole PSUM tile (its out-of-band matrix entries are 0). ---
                # sc3 goes first: its start=True marks the whole 2KB PSUM
                # bank pending-zero (ZERO_REGION_SIZE), so every other chunk
                # accumulates over a zeroed tile with only its reachable band
                EMA_PLAN = [
                    (3, 256, 480, True),
                    (1, 0, 216, False),
                    (0, 0, 96, False),
                    (2, 128, 352, False),
                    (4, 384, BLK, False),
                ]
                ht_sb = apool.tile([128, NDT * WTOK], f16, tag="ht")
                for blk in range(BPW):
                    for dt in range(NDT):
                        ps = ema_ps.tile([128, BLK], f32, tag="ema")
                        for i, (sc, lo, hi, st) in enumerate(EMA_PLAN):
                            off = (blk * (BLK // 128) + sc) * D
                            nc.tensor.matmul(
                                ps[:, lo:hi],
                                x_sb[:, off + dt * 128 : off + (dt + 1) * 128],
                                m_sb[:, sc * BLK + lo : sc * BLK + hi],
                                start=st,
                                stop=(i == len(EMA_PLAN) - 1),
                            )
                        nc.vector.tensor_copy(
                            ht_sb[:, dt * WTOK + blk * BLK : dt * WTOK + (blk + 1) * BLK],
                            ps[:],
                        )

                # --- M1 (blk-major so each block's M2'+softmax overlaps the
                # next block's M1) + M2' + softmax ---
                hid_sb = apool.tile([128, NHT * WTOK], f16, tag="hid")
                for blk in range(BPW):
                    for ht in range(NHT):
                        if nf8:
                            w18_sb = w1pool.tile([128, nf8 * 128], f8, tag="w18")
                            nc.sync.dma_start(w18_sb[:], w1t8[ht, :, :])
                        w116_sb = w1pool.tile(
                            [128, (NKT - nf8) * 128], f16, tag="w116"
                        )
                        half = (NKT - nf8) * 64
                        nc.sync.dma_start(w116_sb[:, :half], w1t16[ht, :, :half])
                        nc.sync.dma_start(w116_sb[:, half:], w1t16[ht, :, half:])
                        ps1 = m1_ps.tile([128, BLK], f32, tag="m1")
                        nmm = nf8 // 2 + (NKT - nf8)
                        mi = 0
                        for j in range(nf8 // 2):
                            nc.tensor.matmul(
                                ps1[:],
                                w18_sb[:, j * 256 : (j + 1) * 256].rearrange(
                                    "p (two m) -> p two m", m=128
                                ),
                                xt8_sb[:, 2 * j * WTOK : (2 * j + 2) * WTOK]
                                .rearrange("p (dt w) -> p dt w", w=WTOK)[
                                    :, :, blk * BLK : (blk + 1) * BLK
                                ],
                                start=(mi == 0),
                                stop=(mi == nmm - 1),
                                perf_mode=DR,
                            )
                            mi += 1
                        for kt in range(nf8, NKT):
                            src = xt16_sb if kt < NDT else ht_sb
                            doff = ((kt - nf8) if kt < NDT else (kt - NDT)) * WTOK
                            nc.tensor.matmul(
                                ps1[:],
                                w116_sb[:, (kt - nf8) * 128 : (kt - nf8 + 1) * 128],
                                src[:, doff + blk * BLK : doff + (blk + 1) * BLK],
                                start=(mi == 0),
                                stop=(mi == nmm - 1),
                            )
                            mi += 1
                        nc.scalar.activation(
                            hid_sb[
                                :, ht * WTOK + blk * BLK : ht * WTOK + (blk + 1) * BLK
                            ],
                            ps1[:],
                            AF.Relu,
                            bias=b1_sb[:, ht : ht + 1],
                            scale=DESCALE,
                        )

                    # --- M2': logits = hid @ Wc^T + bc, tokens on partitions,
                    # then softmax.  Logits are O(1) here so exp runs without
                    # the max-subtraction; sum/recip on Vector, exp + the
                    # final normalize (Copy with scale=rcp) on Scalar. ---
                    bt0 = t0 + blk * BLK
                    ot = smpool.tile([128, (BLK // 128) * E], f32, tag="ot")
                    for tt in range(BLK // 128):
                        ps3 = m3_ps.tile([128, E], f32, tag="m3")
                        for ht in range(NHT):
                            hoff = ht * WTOK + blk * BLK + tt * 128
                            nc.tensor.matmul(
                                ps3[:],
                                hid_sb[:, hoff : hoff + 128],
                                wc_sb[:, ht * E : (ht + 1) * E],
                                start=(ht == 0),
                                stop=(ht == NHT - 1),
                            )
                        lg = smpool.tile([128, E], f32, tag="lg")
                        nc.vector.tensor_add(lg[:], ps3[:], bc_sb[:, :E])
                        ex = smpool.tile([128, E], f32, tag="ex")
                        nc.scalar.activation(ex[:], lg[:], AF.Exp)
                        ssum = smpool.tile([128, 1], f32, tag="ssum")
                        nc.vector.reduce_sum(ssum[:], ex[:], axis=AX.X)
                        rcp = smpool.tile([128, 1], f32, tag="rcp")
                        nc.vector.reciprocal(rcp[:], ssum[:])
                        nc.scalar.activation(
                            ot[:, tt * E : (tt + 1) * E], ex[:], AF.Copy, scale=rcp[:]
                        )
                    # single DMA per block: [4 tok-tiles, 128, E]
                    nc.sync.dma_start(
                        out[bt0 : bt0 + BLK, :].rearrange("(tt p) e -> p tt e", p=128),
                        ot[:].rearrange("p (tt e) -> p tt e", e=E),
                    )

    nc.compile()
    return nc


_prepared = {}


def _prepare_host_inputs(seq, beta_raw, W1, b1, W2, b2, Wr, br, nf8=NF8):
    f8np = ml_dtypes.float8_e4m3
    seq = np.asarray(seq, np.float32)
    beta = 1.0 / (1.0 + np.exp(-np.asarray(beta_raw, np.float64)))
    assert beta.max() - beta.min() < 1e-6, "kernel assumes channel-constant beta"
    b = float(beta[0])
    assert b ** LB < 1e-4, "lookback too short for this beta"

    x = seq[:, : T - 1, :]  # [B, 2047, D]

    # decay matrix: mdec[s, t] = b^((t+LB)-s) for (t+LB)>=s else 0; carries the
    # 2^4 h-scale so the EMA output lands pre-scaled for M1
    s_idx = np.arange(LB + BLK)[:, None]
    t_idx = np.arange(BLK)[None, :]
    expo = (t_idx + LB) - s_idx
    mdec = (XS * np.where(expo >= 0, b ** np.maximum(expo, 0), 0.0)).astype(np.float16)

    W1s = np.asarray(W1, np.float32) * WS
    W2 = np.asarray(W2, np.float32)
    Wr = np.asarray(Wr, np.float32)
    # fold router into predictor layer 2: logits = hid @ (Wr@W2)^T + (br + Wr@b2)
    Wc = (Wr @ W2).astype(np.float32)  # [E, H]
    bc_eff = np.asarray(br, np.float32) + Wr @ np.asarray(b2, np.float32)

    # w1t8[ht, k, j*256 + i*128 + m] = W1s[ht*128+m, (2j+i)*128+k] (DoubleRow pairs)
    if nf8:
        w1x = W1s[:, : nf8 * 128].reshape(NHT, 128, nf8 // 2, 2, 128)
        w1t8 = np.ascontiguousarray(
            w1x.transpose(0, 4, 2, 3, 1).reshape(NHT, 128, nf8 * 128)
        ).astype(f8np)
    # w1t16[ht, k, c*128+m] = W1s[ht*128+m, (nf8+c)*128+k]
    w1r = W1s.reshape(NHT, 128, NKT, 128)[:, :, nf8:, :]
    w1t16 = np.ascontiguousarray(
        w1r.transpose(0, 3, 2, 1).reshape(NHT, 128, (NKT - nf8) * 128)
    ).astype(np.float16)
    b1t = np.ascontiguousarray(np.asarray(b1, np.float32).reshape(NHT, 128).T)
    # wct[p, ht*E+e] = Wc[e, ht*128+p]
    wct = np.ascontiguousarray(
        Wc.T.reshape(NHT, 128, E).transpose(1, 0, 2).reshape(128, NHT * E)
    ).astype(np.float16)
    bcb = np.ascontiguousarray(np.tile(bc_eff[None, :], (128, BLK // 128)))

    shared = dict(mdec=mdec, w1t16=w1t16, b1t=b1t, wct=wct, bcb=bcb)
    if nf8:
        shared["w1t8"] = w1t8
    in_maps = []
    for bi in range(B):
        x_pad = np.zeros((LB + TP, D), np.float16)
        x_pad[LB : LB + T - 1] = x[bi]
        xTs = np.zeros((D, TP), np.float32)
        xTs[:, : T - 1] = x[bi].T * XS
        m = dict(shared)
        m["x_pad"] = x_pad
        if nf8:
            m["xt8"] = np.ascontiguousarray(xTs[: nf8 * 128]).astype(f8np)
        if nf8 < NDT:
            m["xt16"] = np.ascontiguousarray(xTs[nf8 * 128 :]).astype(np.float16)
        in_maps.append(m)
    return in_maps


def kernel(**inputs):
    from concourse import bass_utils

    if NF8 not in _prepared:
        _prepared[NF8] = _build_program()
    nc = _prepared[NF8]
    in_maps = _prepare_host_inputs(**inputs)
    res = bass_utils.run_bass_kernel_spmd(nc, in_maps, core_ids=list(range(B)))
    outs = np.stack([r["out"] for r in res.results], axis=0)  # [B, TP, E]
    return outs[:, : T - 1, :].astype(np.float32)
